# revision 1
# baseline (speedup 1.0000x reference)
"""ArcFace loss on 8 TRN2 NeuronCores (class-dim tensor parallel), v3.

Per core (classes sharded 8 x 12500, padded to 12512):
    cos[n, c] = e_norm[n, :] @ w_norm[c, :]^T   (fp8 DoubleRow, f32 PSUM)
    then per-unit row-sums of exp(64*cos), where a unit is one
    (group, row-tile) PSUM tile [128, W] (W = 2048 x6 + 224).

With 2 PSUM buffers the pipeline paces at the per-unit PSUM-reader
latency, so the design uses two SELF-CONTAINED reader streams that
never wait on each other:
  AA (32 big + 8 small units): ScalarE activation(Exp, accum_out) does
      the exp AND the row-sum in one instruction (~2.6us/unit).
  ZD (16 big units, every 3rd): DVE tensor_scalar computes Schraudolph
      exp bits (p*A + B -> int16, RNE convert measured on HW; the bit
      pattern IS bf16 exp(0.25p)); its reduce_sum over the bf16 bitcast
      is emitted 2 units later (red_lag) so the latency-critical ts
      reads never queue behind a reduce on the DVE.
Engine totals: PE ~94us (fp8 DoubleRow roofline), ACT ~88us, DVE ~90us.
The 6.9MB/rep weight DMA is round-robined over the sync and gpsimd DGE
queues — a single queue was a measured ~15us co-limiter.

Schraudolph bias is mean-zero in log space; its ~1.8% rms element
oscillation averages out in the 12.5k-term sums (tolerance 2e-2,
measured end-to-end error ~1e-4).

Host: target-class terms exactly in f64 (1024 rows), swaps the margined
target logit into the device sum, masked mean over kept rows.
"""

import numpy as np

N, E, C = 1024, 512, 100000
M = 8            # cores
CSH = C // M     # 12500 classes per core
P = 128
NT = N // P      # 8 batch-row tiles
SCALE = 64.0
MARGIN = 0.5
EPS_NORM = 1e-12
CLAMP = 1e-7

FP8_S = 16.0
ACT_SCALE = SCALE / (FP8_S * FP8_S)          # 0.25
CSHP = 12512                                  # padded classes per core
GROUPS = [2048] * 6 + [224]
NG = len(GROUPS)
KP = 2                                        # DoubleRow: 2 contraction chunks

# Schraudolph constants for bf16-bits-as-int16: bits = p*A + B (RNE)
A_SCH = 128.0 * ACT_SCALE * 1.4426950408889634
B_SCH = 128.0 * (127.0 - 0.057304959)         # mean-zero in log space

N_ZD = 16                                     # big units on the DVE stream

_compiled = None
LAST_RESULT = None


def _schedule(n_zd=N_ZD, big_job="AA"):
    """56 units in schedule order (g outer, t inner) with job + out column.

    Big units: n_zd are ZD (DVE Schraudolph stream), the rest `big_job`
    (AA = ACT exp+accum, AD = ACT exp into slab + lagged DVE batch reduce).
    Small (g6) units are always AA.
    """
    jobs_big = []
    done = 0
    for i in range(48):
        want = round(n_zd * (i + 1) / 48.0)
        if want > done:
            jobs_big.append("ZD")
            done += 1
        else:
            jobs_big.append(big_job)
    assert done == n_zd
    units = []
    bi = 0
    counts = {"AA": 0, "ZD": 0, "AD": 0}
    for g, gw in enumerate(GROUPS):
        for t in range(NT):
            job = jobs_big[bi] if g < 6 else "AA"
            if g < 6:
                bi += 1
            u = {"g": g, "t": t, "w": gw, "job": job}
            u["col"] = counts[job]
            counts[job] += 1
            units.append(u)
    return units, counts


UNITS, COUNTS = _schedule()
NAA, NZD = COUNTS["AA"], COUNTS["ZD"]


def _np_in_dtype():
    import concourse.mybir as mybir
    return mybir.dt.np(mybir.dt.float8e4)


def _build(reps=None, units=None, counts=None, ts_only=False,
           red_lag=2, shp_bufs=4, slab_bufs=3, dma_mode="split2h"):
    import contextlib

    import concourse.mybir as mybir
    import concourse.tile as tile
    from concourse import bacc

    if units is None:
        units, counts = UNITS, COUNTS
    naa, nzd, nad = counts["AA"], counts["ZD"], counts["AD"]

    f32 = mybir.dt.float32
    bf16 = mybir.dt.bfloat16
    i16 = mybir.dt.int16
    fin = mybir.dt.float8e4
    EXP = mybir.ActivationFunctionType.Exp
    MULT, ADD = mybir.AluOpType.mult, mybir.AluOpType.add
    DR = mybir.MatmulPerfMode.DoubleRow

    nc = bacc.Bacc("TRN2", target_bir_lowering=False, debug=False, num_devices=M)
    et_d = nc.dram_tensor("et", [KP, P, 2, N], fin, kind="ExternalInput").ap()
    wt_d = nc.dram_tensor("wt", [KP, P, 2, CSHP], fin, kind="ExternalInput").ap()
    outa_d = nc.dram_tensor("outa", [P, max(naa, 1)], f32, kind="ExternalOutput").ap()
    outz_d = nc.dram_tensor("outz", [P, max(nzd, 1)], f32, kind="ExternalOutput").ap()
    outd_d = nc.dram_tensor("outd", [P, max(nad, 1)], f32, kind="ExternalOutput").ap()

    with tile.TileContext(nc) as tc:
        with tc.tile_pool(name="wp", bufs=1) as wp, \
             tc.tile_pool(name="ep", bufs=1) as ep, \
             tc.tile_pool(name="etp", bufs=2) as etp, \
             tc.tile_pool(name="shp", bufs=shp_bufs) as shp, \
             tc.tile_pool(name="slp", bufs=slab_bufs) as slp, \
             tc.tile_pool(name="dmp", bufs=2) as dmp, \
             tc.tile_pool(name="stp", bufs=2) as stp, \
             tc.tile_pool(name="ps", bufs=2, space="PSUM") as pp, \
             (tc.For_i(0, reps, 1,
                       hint_engines=(mybir.EngineType.PE,
                                     mybir.EngineType.Activation))
              if reps else contextlib.nullcontext()):
            # warm the Exp table off the critical path
            warm = ep.tile([P, 1], f32, tag="warm", name="warm")
            nc.vector.memset(warm[:], 0.0)
            warm2 = ep.tile([P, 1], f32, tag="warm2", name="warm2")
            nc.scalar.activation(warm2[:], warm[:], EXP, scale=1.0)

            # weight DMA through multiple DGE queues: one queue saturates
            # well below the per-rep 6.9MB need
            halved = dma_mode == "split2h"
            dma_engs = {"single": [nc.sync],
                        "split2": [nc.sync, nc.gpsimd],
                        "split2h": [nc.sync, nc.gpsimd],
                        "split3": [nc.sync, nc.gpsimd, nc.scalar],
                        "none": None}[dma_mode]

            def load(t, src, di=[0]):
                if dma_engs is None:   # diagnostic: no-DMA ceiling
                    nc.gpsimd.memset(t[:], 0.0)
                    return
                if halved:
                    h = t.shape[-1] // 2
                    for dst_h, src_h in ((t[:, :, :h], src[:, :, :h]),
                                         (t[:, :, h:], src[:, :, h:])):
                        e = dma_engs[di[0] % len(dma_engs)]
                        di[0] += 1
                        e.dma_start(dst_h, src_h)
                else:
                    e = dma_engs[di[0] % len(dma_engs)]
                    di[0] += 1
                    e.dma_start(t[:], src)

            et = []
            for k in range(KP):
                t = etp.tile([P, 2, N], fin, tag=f"et{k}", name=f"et{k}")
                load(t, et_d[k])
                et.append(t)
            wt = []
            col = 0
            for g, gw in enumerate(GROUPS):
                tk = []
                for k in range(KP):
                    t = wp.tile([P, 2, gw], fin, tag=f"w{g}_{k}", name=f"w{g}_{k}")
                    load(t, wt_d[k, :, :, col:col + gw])
                    tk.append(t)
                wt.append(tk)
                col += gw

            stats_a = stp.tile([P, max(naa, 1)], f32, tag="sta", name="sta")
            stats_z = stp.tile([P, max(nzd, 1)], f32, tag="stz", name="stz")
            stats_d = stp.tile([P, max(nad, 1)], f32, tag="std", name="std")
            if nzd == 0 or ts_only:
                nc.vector.memset(stats_z[:], 0.0)
            if naa == 0:
                nc.vector.memset(stats_a[:], 0.0)
            if nad == 0:
                nc.vector.memset(stats_d[:], 0.0)

            slab, scur, sbase = None, 0, 0
            pending = []  # (due_unit_idx, emit_fn) delayed DVE reduces
            for ui, u in enumerate(units):
                while pending and pending[0][0] <= ui:
                    pending.pop(0)[1]()
                g, t, w, job = u["g"], u["t"], u["w"], u["job"]
                ps = pp.tile([P, 2048], f32, tag="ps", name=f"ps{g}_{t}")
                nsub = (w + 511) // 512
                for k in range(KP):
                    for j in range(nsub):
                        jw = min(512, w - j * 512)
                        nc.tensor.matmul(
                            ps[:, j * 512:j * 512 + jw],
                            et[k][:, :, t * P:(t + 1) * P],
                            wt[g][k][:, :, j * 512:j * 512 + jw],
                            start=(k == 0), stop=(k == KP - 1),
                            perf_mode=DR,
                        )
                if job == "AA":
                    dump = dmp.tile([P, 2048], bf16, tag="dump",
                                    name=f"dump{g}_{t}")
                    nc.scalar.activation(
                        dump[:, :w], ps[:, :w], EXP, scale=ACT_SCALE,
                        accum_out=stats_a[:, u["col"]:u["col"] + 1])
                elif job == "AD":
                    if scur == 0:
                        sbase = u["col"]
                        slab = slp.tile([P, 4, 2048], bf16, tag="slab",
                                        name=f"slab{sbase}")
                    nc.scalar.activation(
                        slab[:, scur:scur + 1, :], ps[:, :w], EXP,
                        scale=ACT_SCALE)
                    scur += 1
                    if scur == 4:
                        def emit_d(slab=slab, sbase=sbase):
                            nc.vector.reduce_sum(
                                stats_d[:, sbase:sbase + 4], slab[:],
                                axis=mybir.AxisListType.X)
                        if red_lag == 0:
                            emit_d()
                        else:
                            pending.append((ui + red_lag, emit_d))
                        scur = 0
                else:  # ZD
                    sch = shp.tile([P, 2048], i16, tag="sch",
                                   name=f"sch{g}_{t}")
                    nc.vector.tensor_scalar(
                        sch[:, :w], ps[:, :w], A_SCH, B_SCH, MULT, ADD)
                    if not ts_only:
                        def emit(sch=sch, w=w, col=u["col"]):
                            nc.vector.reduce_sum(
                                stats_z[:, col:col + 1],
                                sch[:, :w].bitcast(bf16),
                                axis=mybir.AxisListType.X)
                        if red_lag == 0:
                            emit()
                        else:
                            pending.append((ui + red_lag, emit))
            for _, fn in pending:
                fn()
            if scur != 0:
                nc.vector.reduce_sum(
                    stats_d[:, sbase:sbase + scur],
                    slab[:, :scur, :], axis=mybir.AxisListType.X)

            nc.sync.dma_start(outa_d[:, :], stats_a[:])
            nc.sync.dma_start(outz_d[:, :], stats_z[:])
            nc.sync.dma_start(outd_d[:, :], stats_d[:])

    nc.compile()
    return nc


def _prep_operands(e, w):
    """Normalize rows, pre-scale, quantize, lay out [KP, P, 2, cols]."""
    dt = _np_in_dtype()
    s = FP8_S
    wn = (w * (s / np.maximum(np.sqrt(np.einsum('ij,ij->i', w, w)), EPS_NORM))[:, None]).astype(dt)
    en = (e * (s / np.maximum(np.sqrt(np.einsum('ij,ij->i', e, e)), EPS_NORM))[:, None]).astype(dt)

    def lay(xT, cols):  # xT: [E, cols] -> [KP, P, 2, cols]
        return np.ascontiguousarray(
            xT.reshape(KP, 2, P, cols).transpose(0, 2, 1, 3))

    et_arr = lay(np.ascontiguousarray(en.T), N)
    shards = []
    for i in range(M):
        blk = wn[i * CSH:(i + 1) * CSH]
        bT = np.zeros((E, CSHP), dt)
        bT[:, :CSH] = blk.T
        shards.append(lay(bT, CSHP))
    return et_arr, shards


def kernel(embedding, ground_truth, weight):
    global _compiled, LAST_RESULT
    import os
    os.environ["BASS_NEVER_TRACE"] = "1"
    from concourse.bass_utils import run_bass_kernel_spmd

    e = np.ascontiguousarray(np.asarray(embedding, dtype=np.float32))
    w = np.ascontiguousarray(np.asarray(weight, dtype=np.float32))
    gt = np.asarray(ground_truth).astype(np.int64)

    et_arr, shards = _prep_operands(e, w)
    in_maps = [{"et": et_arr, "wt": shards[i]} for i in range(M)]

    if _compiled is None:
        _compiled = _build()
    LAST_RESULT = run_bass_kernel_spmd(_compiled, in_maps, core_ids=list(range(M)))

    # ---- host combine (f64) ----
    maps = {k: np.asarray([u["t"] for u in UNITS if u["job"] == k] or [-1])
            for k in ("AA", "ZD", "AD")}
    outs = {"AA": "outa", "ZD": "outz", "AD": "outd"}
    S = np.zeros(N, np.float64)
    for r in LAST_RESULT.results:
        for k, oname in outs.items():
            o = r[oname].astype(np.float64)
            tm = maps[k]
            if tm[0] == -1:
                continue
            for t in range(NT):
                sl = slice(t * P, (t + 1) * P)
                S[sl] += o[:, tm == t].sum(axis=1)
    S -= float(M * (CSHP - CSH))   # zero-pad cols: exp(0)=1 each, in AA units

    # exact target-class terms
    e64 = e.astype(np.float64)
    en64 = e64 / np.maximum(np.sqrt((e64 * e64).sum(1, keepdims=True)), EPS_NORM)
    wg = w[gt].astype(np.float64)
    wg /= np.maximum(np.sqrt((wg * wg).sum(1, keepdims=True)), EPS_NORM)
    cos_gt = np.clip((en64 * wg).sum(1), -1.0 + CLAMP, 1.0 - CLAMP)
    keep = (np.arccos(cos_gt) + MARGIN) <= np.pi
    tgt = SCALE * (cos_gt * np.cos(MARGIN) - np.sqrt(1.0 - cos_gt * cos_gt) * np.sin(MARGIN))

    S_corr = S - np.exp(SCALE * cos_gt) + np.exp(tgt)
    nll = np.log(S_corr) - tgt
    loss = (nll * keep).sum() / max(keep.sum(), 1.0)
    return np.float32(loss)

